# revision 16
# baseline (speedup 1.0000x reference)
"""Distributed contrastive loss kernel for 8 Trainium2 NeuronCores.

loss = mean_i( logsumexp_j(f1n_i . f2n_j / T) - (f1n_i . f2n_i) / T )
with f1n/f2n the L2-row-normalized feature matrices, N=16384, D=512.

v2 design (fp8 DoubleRow):
- f1 rows sharded 8 ways (2048/core); f2 rows likewise.
- f2 path first: per 512-row chunk, normalize rows on DVE (inv2*32 folded into
  an fp8 cast), PE-transpose to [512, 512] fp8 blocks, AllGather the chunk.
  Four chunked AllGathers pipeline with the rest of prep + main loop.
- f1 path: raw rows cast to fp8, PE-transposed into a [128, 4, 2048] weight
  tile (k-subtile layout for DoubleRow). inv1/T/32 folded into the Exp scale.
- Main loop: per (chunk g, rank-half h, m-tile) unit, 8 DoubleRow fp8 matmuls
  (2 k-pairs x 4 ranks) accumulate a [128, 2048] f32 logits tile in PSUM,
  then ONE ScalarE Exp with per-partition scale = inv1/(T*32) and fused
  accum_out produces row-wise exp-sums. Logits never leave PSUM.
- diag from an elementwise f32 dot of the local shards; logsumexp via Ln of
  accumulated row sums; per-core partial reduced with a ones-matmul. Host
  sums 8 partials / N.
"""

import os
from contextlib import ExitStack
from functools import lru_cache

import numpy as np

import concourse.bass as bass
import concourse.mybir as mybir
import concourse.tile as tile
from concourse.bass_utils import run_bass_kernel_spmd
from concourse.masks import make_identity

# Problem shape (hardcoded per contest rules).
N = 16384
D = 512
N_CORES = 8
M_LOCAL = N // N_CORES  # 2048 rows per core
TEMP = 0.07

P = 128                 # SBUF partitions
DC = D // P             # 4 contraction chunks of 128
MT = M_LOCAL // P       # 16 m-tiles (and f2-shard tiles) per core
GW = 512                # AllGather chunk width (f2 rows per chunk)
NG = M_LOCAL // GW      # 4 AllGather chunks
GT = GW // P            # 4 f2-shard tiles per chunk
EXPW = 2048             # exp tile width = 4 ranks x 512 cols
F2S = 32.0              # power-of-2 scale folded into f2n fp8 cast
F32 = mybir.dt.float32
BF16 = mybir.dt.bfloat16
F8 = mybir.dt.float8e4
AF = mybir.ActivationFunctionType
ALU = mybir.AluOpType
DR = mybir.MatmulPerfMode.DoubleRow

# Module-level stash for the last run's profile (read by test.py).
LAST_EXEC_TIME_NS = None


def _install_ntff_hook():
    """Provide antenv.axon_hooks (missing from this image) so that
    run_bass_kernel_spmd(trace=True) can capture NTFF profiles via the
    axon PJRT .so. Mirrors trn_agent_boot.trn_boot._ntff_profile_via_ctypes."""
    import contextlib
    import ctypes
    import sys
    import types

    try:
        import antenv.axon_hooks  # noqa: F401

        return
    except ImportError:
        pass

    so_path = "/opt/axon/libaxon_pjrt.so"
    hook = None
    try:
        lib = ctypes.CDLL(so_path)
        if hasattr(lib, "axon_start_nrt_profile"):
            lib.axon_start_nrt_profile.argtypes = [
                ctypes.POINTER(ctypes.c_int64),
                ctypes.c_size_t,
            ]
            lib.axon_start_nrt_profile.restype = ctypes.c_int64
            lib.axon_stop_nrt_profile.argtypes = [ctypes.c_char_p]
            lib.axon_stop_nrt_profile.restype = ctypes.c_int64

            @contextlib.contextmanager
            def _hook(output_dir, device_ids):
                import jax

                jax.devices()
                if device_ids:
                    ids = (ctypes.c_int64 * len(device_ids))(*device_ids)
                    rc = lib.axon_start_nrt_profile(ids, len(device_ids))
                else:
                    rc = lib.axon_start_nrt_profile(None, 0)
                if rc != 0:
                    raise RuntimeError(f"axon_start_nrt_profile rc={rc}")
                try:
                    yield
                finally:
                    n = lib.axon_stop_nrt_profile(str(output_dir).encode())
                    print(f"profile: {n} file(s) written to {output_dir}", file=sys.stderr)

            hook = _hook
    except OSError:
        pass

    import antenv

    mod = types.ModuleType("antenv.axon_hooks")
    _state = {"hook": hook}
    mod.set_axon_ntff_profile_hook = lambda h: _state.__setitem__("hook", h)
    mod.get_axon_ntff_profile_hook = lambda: _state["hook"]
    sys.modules["antenv.axon_hooks"] = mod
    antenv.axon_hooks = mod

    # Artifact upload needs external storage creds; make it a no-op.
    import concourse.bass_utils as _bu

    _bu.upload_artifacts = lambda tmpdir: f"local:{tmpdir}"


def _build_bass():
    nc = bass.Bass(num_devices=N_CORES, debug=False)

    f1s = nc.dram_tensor("f1s", [M_LOCAL, D], F32, kind="ExternalInput")
    f2o = nc.dram_tensor("f2o", [M_LOCAL, D], F32, kind="ExternalInput")
    out = nc.dram_tensor("out", [1, 1], F32, kind="ExternalOutput")

    inv_temp = 1.0 / TEMP

    with tile.TileContext(nc) as tc, ExitStack() as ctx:
        consts = ctx.enter_context(tc.tile_pool(name="consts", bufs=1))
        resident = ctx.enter_context(tc.tile_pool(name="resident", bufs=1))
        work = ctx.enter_context(tc.tile_pool(name="work", bufs=3))
        stats = ctx.enter_context(tc.tile_pool(name="stats", bufs=4))
        rhsp = ctx.enter_context(tc.tile_pool(name="rhsp", bufs=3))
        psum = ctx.enter_context(tc.tile_pool(name="psum", bufs=2, space="PSUM"))
        dram = ctx.enter_context(tc.tile_pool(name="dram", bufs=1, space="DRAM"))

        identity = consts.tile([P, P], BF16)
        make_identity(nc, identity)
        ones_col = consts.tile([P, 1], F32)
        nc.vector.memset(ones_col, 1.0)


        # Residents.
        f1p = resident.tile([P, DC, M_LOCAL], F8, name="f1p")
        f2T = [resident.tile([P, DC, GW], F8, name=f"f2T{g}") for g in range(NG)]
        x1res = [resident.tile([P, D], F32, name=f"x1r{t}") for t in range(MT)]
        x2res = [resident.tile([P, D], F32, name=f"x2r{t}") for t in range(MT)]
        ss1g = resident.tile([P, MT], F32, name="ss1g")
        ss2g = resident.tile([P, MT], F32, name="ss2g")
        ss12g = resident.tile([P, MT], F32, name="ss12g")
        diag_raw = resident.tile([P, MT], F32, name="diag_raw")
        diag = resident.tile([P, MT], F32, name="diag")
        inv1g = resident.tile([P, MT], F32, name="inv1g")
        inv2g = resident.tile([P, MT], F32, name="inv2g")
        i232 = resident.tile([P, MT], F32, name="i232")
        scale1g = resident.tile([P, MT], F32, name="scale1g")
        rowsums = resident.tile([P, MT, NG * 2], F32, name="rowsums")
        losses = resident.tile([P, MT], F32, name="losses")

        # AllGather bounce buffers, one pair per chunk.
        agin = [dram.tile([DC * P, GW], F8, name=f"agin{g}") for g in range(NG)]
        agout = [
            dram.tile([N_CORES * DC * P, GW], F8, name=f"agout{g}",
                      addr_space="Shared")
            for g in range(NG)
        ]

        def sumsq_col(x, col, tag):
            """sum(x^2) per row of [P, D] tile -> col ([P,1] slice), on ScalarE
            (Square is in the natural_log_exp table set; DVE stays free)."""
            sqo = work.tile([P, D], BF16, tag="sqo", bufs=2, name=f"sqo{tag}")
            nc.scalar.activation(sqo, x, AF.Square, accum_out=col)

        def rsqrt_quarter(dst, src, gsl, tag):
            """dst[:, gsl] = 1/sqrt(src[:, gsl]) via exp(-0.5*ln(.)) - one table set."""
            ln = stats.tile([P, GT], F32, tag=f"ln{tag}", name=f"ln{tag}")
            nc.scalar.activation(ln, src[:, gsl], AF.Ln)
            nc.scalar.activation(dst[:, gsl], ln, AF.Exp, scale=-0.5)

        def transpose_tile(src_bf16, dst, dst_col_off):
            """PE-transpose a [P, D] bf16 tile into dst[:, c, off:off+P] fp8
            via one packed [P, D] psum tile and one wide strided copy."""
            tp = psum.tile([P, D], BF16, tag="ps", name="tp")
            for c in range(DC):
                nc.tensor.matmul(
                    tp[:, c * P : (c + 1) * P],
                    lhsT=src_bf16[:, c * P : (c + 1) * P],
                    rhs=identity,
                    is_transpose=True,
                    start=(c == 0),
                    stop=(c == DC - 1),
                )
            nc.vector.tensor_copy(dst[:, :, dst_col_off : dst_col_off + P], tp)

        # ---- All input loads upfront: the sync queue must never block them --
        for t in range(MT):
            nc.sync.dma_start(out=x2res[t], in_=f2o[t * P : (t + 1) * P, :])
        for t in range(MT):
            nc.sync.dma_start(out=x1res[t], in_=f1s[t * P : (t + 1) * P, :])

        # ---- Phase F2: normalize + transpose + AllGather, chunk by chunk ----
        def f2_group(g):
            gsl = slice(g * GT, (g + 1) * GT)
            for t in range(g * GT, (g + 1) * GT):
                sumsq_col(x2res[t], ss2g[:, t : t + 1], "2")
            rsqrt_quarter(inv2g, ss2g, gsl, "2")
            nc.vector.tensor_scalar_mul(i232[:, gsl], inv2g[:, gsl], F2S)
            for t in range(g * GT, (g + 1) * GT):
                x2c = work.tile([P, D], BF16, tag="x2c", name="x2c")
                nc.vector.tensor_scalar_mul(x2c, x2res[t], i232[:, t : t + 1])
                transpose_tile(x2c, f2T[g], (t - g * GT) * P)
            for c in range(DC):
                nc.sync.dma_start(
                    out=agin[g][c * P : (c + 1) * P, :], in_=f2T[g][:, c, :]
                )
            nc.gpsimd.collective_compute(
                "AllGather",
                ALU.bypass,
                replica_groups=[list(range(N_CORES))],
                ins=[agin[g][:, :].opt()],
                outs=[agout[g][:, :].opt()],
            )

        for g in range(NG):
            f2_group(g)

        # ---- Phase F1: cast, transpose, stats, diag -------------------------
        def f1_group(g):
            gsl = slice(g * GT, (g + 1) * GT)
            for t in range(g * GT, (g + 1) * GT):
                x1c = work.tile([P, D], BF16, tag="x1c", name="x1c")
                nc.vector.tensor_copy(x1c, x1res[t])
                transpose_tile(x1c, f1p, t * P)
                sumsq_col(x1res[t], ss1g[:, t : t + 1], "1")
                # diag via polarization: 2*x1.x2 = ||x1+x2||^2 - ||x1||^2 - ||x2||^2
                x12 = work.tile([P, D], F32, tag="x12", bufs=2, name="x12")
                nc.vector.tensor_tensor(x12, x1res[t], x2res[t], ALU.add)
                sumsq_col(x12, ss12g[:, t : t + 1], "12")
            rsqrt_quarter(inv1g, ss1g, gsl, "1")
            nc.vector.tensor_scalar_mul(
                scale1g[:, gsl], inv1g[:, gsl], inv_temp / F2S
            )
            # diag_raw = 0.5*(ss12 - ss1 - ss2); diag = diag_raw * inv1 * inv2.
            nc.vector.tensor_tensor(
                diag_raw[:, gsl], ss12g[:, gsl], ss1g[:, gsl], ALU.subtract
            )
            nc.vector.tensor_tensor(
                diag_raw[:, gsl], diag_raw[:, gsl], ss2g[:, gsl], ALU.subtract
            )
            nc.vector.tensor_tensor(
                diag[:, gsl], diag_raw[:, gsl], inv1g[:, gsl], ALU.mult
            )
            nc.vector.tensor_tensor(
                diag[:, gsl], diag[:, gsl], inv2g[:, gsl], ALU.mult
            )
            nc.vector.tensor_scalar_mul(diag[:, gsl], diag[:, gsl], 0.5)

        for g in range(NG):
            f1_group(g)

        # ---- Phase 3: fused logits -> exp(scale=inv1/(T*32)) -> row-sums ----
        def main_group(g):
            for h in range(2):
                rh = rhsp.tile([P, DC, EXPW], F8, tag="rh", name="rh")
                for j in range(4):
                    r = 4 * h + j
                    for c in range(DC):
                        nc.sync.dma_start(
                            out=rh[:, c, j * GW : (j + 1) * GW],
                            in_=agout[g][r * D + c * P : r * D + (c + 1) * P, :],
                        )
                for mt in range(MT):
                    ps = psum.tile([P, EXPW], F32, tag="ps", name="ps")
                    for cp in range(2):
                        lhsT = f1p[:, 2 * cp : 2 * cp + 2, mt * P : (mt + 1) * P]
                        for j in range(4):
                            nc.tensor.matmul(
                                ps[:, j * GW : (j + 1) * GW],
                                lhsT=lhsT,
                                rhs=rh[:, 2 * cp : 2 * cp + 2, j * GW : (j + 1) * GW],
                                start=(cp == 0),
                                stop=(cp == 1),
                                perf_mode=DR,
                            )
                    # Split the row-sum load between the two near-saturated
                    # engines: even units pay the 182ns accumulator drain on
                    # ACT, odd units pay a ~2us reduce on DVE. Either engine
                    # alone would pace the loop; alternating keeps both under.
                    ex = work.tile([P, EXPW], BF16, tag="ex", bufs=4, name="ex")
                    rs_col = rowsums[:, mt, 2 * g + h : 2 * g + h + 1]
                    if mt % 2 == 0:
                        nc.scalar.activation(
                            ex, ps, AF.Exp,
                            scale=scale1g[:, mt : mt + 1],
                            accum_out=rs_col,
                        )
                    else:
                        nc.scalar.activation(
                            ex, ps, AF.Exp, scale=scale1g[:, mt : mt + 1]
                        )
                        nc.vector.reduce_sum(
                            rs_col, ex, axis=mybir.AxisListType.X
                        )

        for g in range(NG):
            main_group(g)

        # ---- Phase 4: logsumexp, subtract diag, reduce -----------------------
        s16 = stats.tile([P, MT], F32, tag="s16", name="s16")
        nc.vector.reduce_sum(s16, rowsums, axis=mybir.AxisListType.X)
        lse16 = stats.tile([P, MT], F32, tag="lse16", name="lse16")
        nc.scalar.activation(lse16, s16, AF.Ln)
        # losses = lse - diag/T = (diag * -1/T) + lse
        nc.vector.scalar_tensor_tensor(
            out=losses,
            in0=diag,
            scalar=-inv_temp,
            in1=lse16,
            op0=ALU.mult,
            op1=ALU.add,
        )

        loss_col = stats.tile([P, 1], F32, tag="lc", name="loss_col")
        nc.vector.reduce_sum(loss_col, losses, axis=mybir.AxisListType.X)
        fin = psum.tile([1, 1], F32, tag="ps", name="fin")
        nc.tensor.matmul(fin, lhsT=loss_col, rhs=ones_col, start=True, stop=True)
        res = stats.tile([1, 1], F32, tag="res", name="res")
        nc.any.tensor_copy(res, fin)
        nc.sync.dma_start(out=out[:, :], in_=res)

    return nc


_WAIT_EXEMPT = ("InstCall",)


def _legalize_sync_waits(nc, limit=1):
    """Walrus codegen rejects instructions carrying more than ~1 embedded
    semaphore wait ("Too many sync wait commands"). Move excess waits onto
    injected same-engine NoOps (one wait each) ahead of the instruction —
    semantically identical (the engine blocks on the NoOps first)."""
    n_split = 0
    for b in nc.m.functions[0].blocks:
        insts = b.instructions
        out = []
        changed = False
        for ins in insts:
            si = ins.sync_info
            tname = type(ins).__name__
            if (
                si is not None
                and len(si.on_wait) > limit
                and tname not in _WAIT_EXEMPT
            ):
                waits = list(si.on_wait)
                keep, excess = waits[:limit], waits[limit:]
                for j, w in enumerate(excess):
                    noop = mybir.InstNoOp(name=f"{ins.name}-ws{j}", ins=[], outs=[])
                    noop.engine = ins.engine
                    noop.sync_info = mybir.SyncInfo(on_wait=[w], on_update=[])
                    out.append(noop)
                ins.sync_info = mybir.SyncInfo(
                    on_wait=keep, on_update=list(si.on_update)
                )
                n_split += 1
                changed = True
            out.append(ins)
        if changed:
            b.instructions = out
    return n_split


def _maybe_patch_ldw_opt():
    """KERNEL_LDW_OPT=1 flips walrus --enable-ldw-opt to true (dedupes /
    optimizes LDWEIGHTS); A/B experiment, correctness-checked by the rel-err
    gate."""
    if not int(os.environ.get("KERNEL_LDW_OPT", "0")):
        return
    import concourse.bass_utils as bu

    if getattr(bu.run_command, "_ldw_patched", False):
        return
    orig = bu.run_command

    def run2(cmd, **kw):
        cmd = [
            "--enable-ldw-opt=true" if c == "--enable-ldw-opt=false" else c
            for c in cmd
        ]
        return orig(cmd, **kw)

    run2._ldw_patched = True
    bu.run_command = run2


@lru_cache(maxsize=1)
def _get_nc():
    _maybe_patch_ldw_opt()
    nc = _build_bass()
    _legalize_sync_waits(nc)
    return nc


def kernel(features1, features2):
    global LAST_EXEC_TIME_NS
    f1 = np.ascontiguousarray(np.asarray(features1, dtype=np.float32))
    f2 = np.ascontiguousarray(np.asarray(features2, dtype=np.float32))
    assert f1.shape == (N, D) and f2.shape == (N, D)

    in_maps = []
    for i in range(N_CORES):
        sl = slice(i * M_LOCAL, (i + 1) * M_LOCAL)
        in_maps.append(
            {
                "f1s": np.ascontiguousarray(f1[sl]),
                "f2o": np.ascontiguousarray(f2[sl]),
            }
        )

    nc = _get_nc()
    trace = bool(int(os.environ.get("KERNEL_TRACE", "0")))
    if trace:
        _install_ntff_hook()
    tmpdir = os.environ.get("KERNEL_TRACE_DIR") or None
    r = run_bass_kernel_spmd(
        nc, in_maps, list(range(N_CORES)), trace=trace, tmpdir=tmpdir
    )
    LAST_EXEC_TIME_NS = r.exec_time_ns

    total = sum(float(r.results[i]["out"][0, 0]) for i in range(N_CORES))
    return np.float32(total / N)


# revision 18
# speedup vs baseline: 1.0404x; 1.0404x over previous
"""Distributed contrastive loss kernel for 8 Trainium2 NeuronCores.

loss = mean_i( logsumexp_j(f1n_i . f2n_j / T) - (f1n_i . f2n_i) / T )
with f1n/f2n the L2-row-normalized feature matrices, N=16384, D=512.

v2 design (fp8 DoubleRow):
- f1 rows sharded 8 ways (2048/core); f2 rows likewise.
- f2 path first: per 512-row chunk, normalize rows on DVE (inv2*32 folded into
  an fp8 cast), PE-transpose to [512, 512] fp8 blocks, AllGather the chunk.
  Four chunked AllGathers pipeline with the rest of prep + main loop.
- f1 path: raw rows cast to fp8, PE-transposed into a [128, 4, 2048] weight
  tile (k-subtile layout for DoubleRow). inv1/T/32 folded into the Exp scale.
- Main loop: per (chunk g, rank-half h, m-tile) unit, 8 DoubleRow fp8 matmuls
  (2 k-pairs x 4 ranks) accumulate a [128, 2048] f32 logits tile in PSUM,
  then ONE ScalarE Exp with per-partition scale = inv1/(T*32) and fused
  accum_out produces row-wise exp-sums. Logits never leave PSUM.
- diag from an elementwise f32 dot of the local shards; logsumexp via Ln of
  accumulated row sums; per-core partial reduced with a ones-matmul. Host
  sums 8 partials / N.
"""

import os
from contextlib import ExitStack
from functools import lru_cache

import numpy as np

import concourse.bass as bass
import concourse.mybir as mybir
import concourse.tile as tile
from concourse.bass_utils import run_bass_kernel_spmd
from concourse.masks import make_identity

# Problem shape (hardcoded per contest rules).
N = 16384
D = 512
N_CORES = 8
M_LOCAL = N // N_CORES  # 2048 rows per core
TEMP = 0.07

P = 128                 # SBUF partitions
DC = D // P             # 4 contraction chunks of 128
MT = M_LOCAL // P       # 16 m-tiles (and f2-shard tiles) per core
GW = 512                # AllGather chunk width (f2 rows per chunk)
NG = M_LOCAL // GW      # 4 AllGather chunks
GT = GW // P            # 4 f2-shard tiles per chunk
EXPW = 2048             # exp tile width = 4 ranks x 512 cols
F2S = 32.0              # power-of-2 scale folded into f2n fp8 cast
F32 = mybir.dt.float32
BF16 = mybir.dt.bfloat16
F8 = mybir.dt.float8e4
AF = mybir.ActivationFunctionType
ALU = mybir.AluOpType
DR = mybir.MatmulPerfMode.DoubleRow

# Module-level stash for the last run's profile (read by test.py).
LAST_EXEC_TIME_NS = None


def _install_ntff_hook():
    """Provide antenv.axon_hooks (missing from this image) so that
    run_bass_kernel_spmd(trace=True) can capture NTFF profiles via the
    axon PJRT .so. Mirrors trn_agent_boot.trn_boot._ntff_profile_via_ctypes."""
    import contextlib
    import ctypes
    import sys
    import types

    try:
        import antenv.axon_hooks  # noqa: F401

        return
    except ImportError:
        pass

    so_path = "/opt/axon/libaxon_pjrt.so"
    hook = None
    try:
        lib = ctypes.CDLL(so_path)
        if hasattr(lib, "axon_start_nrt_profile"):
            lib.axon_start_nrt_profile.argtypes = [
                ctypes.POINTER(ctypes.c_int64),
                ctypes.c_size_t,
            ]
            lib.axon_start_nrt_profile.restype = ctypes.c_int64
            lib.axon_stop_nrt_profile.argtypes = [ctypes.c_char_p]
            lib.axon_stop_nrt_profile.restype = ctypes.c_int64

            @contextlib.contextmanager
            def _hook(output_dir, device_ids):
                import jax

                jax.devices()
                if device_ids:
                    ids = (ctypes.c_int64 * len(device_ids))(*device_ids)
                    rc = lib.axon_start_nrt_profile(ids, len(device_ids))
                else:
                    rc = lib.axon_start_nrt_profile(None, 0)
                if rc != 0:
                    raise RuntimeError(f"axon_start_nrt_profile rc={rc}")
                try:
                    yield
                finally:
                    n = lib.axon_stop_nrt_profile(str(output_dir).encode())
                    print(f"profile: {n} file(s) written to {output_dir}", file=sys.stderr)

            hook = _hook
    except OSError:
        pass

    import antenv

    mod = types.ModuleType("antenv.axon_hooks")
    _state = {"hook": hook}
    mod.set_axon_ntff_profile_hook = lambda h: _state.__setitem__("hook", h)
    mod.get_axon_ntff_profile_hook = lambda: _state["hook"]
    sys.modules["antenv.axon_hooks"] = mod
    antenv.axon_hooks = mod

    # Artifact upload needs external storage creds; make it a no-op.
    import concourse.bass_utils as _bu

    _bu.upload_artifacts = lambda tmpdir: f"local:{tmpdir}"


def _build_bass():
    nc = bass.Bass(num_devices=N_CORES, debug=False)

    f1s = nc.dram_tensor("f1s", [M_LOCAL, D], F32, kind="ExternalInput")
    f2o = nc.dram_tensor("f2o", [M_LOCAL, D], F32, kind="ExternalInput")
    out = nc.dram_tensor("out", [1, 1], F32, kind="ExternalOutput")

    inv_temp = 1.0 / TEMP

    with tile.TileContext(nc) as tc, ExitStack() as ctx:
        consts = ctx.enter_context(tc.tile_pool(name="consts", bufs=1))
        resident = ctx.enter_context(tc.tile_pool(name="resident", bufs=1))
        work = ctx.enter_context(tc.tile_pool(name="work", bufs=3))
        stats = ctx.enter_context(tc.tile_pool(name="stats", bufs=4))
        rhsp = ctx.enter_context(tc.tile_pool(name="rhsp", bufs=2))
        psum = ctx.enter_context(tc.tile_pool(name="psum", bufs=2, space="PSUM"))
        dram = ctx.enter_context(tc.tile_pool(name="dram", bufs=1, space="DRAM"))

        identity = consts.tile([P, P], BF16)
        make_identity(nc, identity)
        ones_col = consts.tile([P, 1], F32)
        nc.vector.memset(ones_col, 1.0)


        # Residents.
        f1p = resident.tile([P, DC, M_LOCAL], F8, name="f1p")
        f2T = [resident.tile([P, DC, GW], F8, name=f"f2T{g}") for g in range(NG)]
        x1res = [resident.tile([P, D], F32, name=f"x1r{t}") for t in range(MT)]
        x2res = [resident.tile([P, D], F32, name=f"x2r{t}") for t in range(MT)]
        ss1g = resident.tile([P, MT], F32, name="ss1g")
        ss2g = resident.tile([P, MT], F32, name="ss2g")
        ss12g = resident.tile([P, MT], F32, name="ss12g")
        diag_raw = resident.tile([P, MT], F32, name="diag_raw")
        diag = resident.tile([P, MT], F32, name="diag")
        inv1g = resident.tile([P, MT], F32, name="inv1g")
        inv2g = resident.tile([P, MT], F32, name="inv2g")
        i232 = resident.tile([P, MT], F32, name="i232")
        scale1g = resident.tile([P, MT], F32, name="scale1g")
        rowsums = resident.tile([P, MT, NG * 2], F32, name="rowsums")
        losses = resident.tile([P, MT], F32, name="losses")

        # AllGather bounce buffers, one pair per chunk.
        agin = [dram.tile([DC * P, GW], F8, name=f"agin{g}") for g in range(NG)]
        agout = [
            dram.tile([N_CORES * DC * P, GW], F8, name=f"agout{g}",
                      addr_space="Shared")
            for g in range(NG)
        ]

        def sumsq_col(x, col, tag):
            """sum(x^2) per row of [P, D] tile -> col ([P,1] slice), on ScalarE
            (Square is in the natural_log_exp table set; DVE stays free)."""
            sqo = work.tile([P, D], BF16, tag="sqo", bufs=2, name=f"sqo{tag}")
            nc.scalar.activation(sqo, x, AF.Square, accum_out=col)

        def rsqrt_quarter(dst, src, gsl, tag):
            """dst[:, gsl] = 1/sqrt(src[:, gsl]) via exp(-0.5*ln(.)) - one table set."""
            ln = stats.tile([P, GT], F32, tag=f"ln{tag}", name=f"ln{tag}")
            nc.scalar.activation(ln, src[:, gsl], AF.Ln)
            nc.scalar.activation(dst[:, gsl], ln, AF.Exp, scale=-0.5)

        def transpose_tile(src_bf16, dst, dst_col_off):
            """PE-transpose a [P, D] bf16 tile into dst[:, c, off:off+P] fp8
            via one packed [P, D] psum tile and one wide strided copy."""
            tp = psum.tile([P, D], BF16, tag="ps", name="tp")
            for c in range(DC):
                nc.tensor.matmul(
                    tp[:, c * P : (c + 1) * P],
                    lhsT=src_bf16[:, c * P : (c + 1) * P],
                    rhs=identity,
                    is_transpose=True,
                    start=(c == 0),
                    stop=(c == DC - 1),
                )
            nc.vector.tensor_copy(dst[:, :, dst_col_off : dst_col_off + P], tp)

        # ---- All input loads upfront: the sync queue must never block them --
        for t in range(MT):
            nc.sync.dma_start(out=x2res[t], in_=f2o[t * P : (t + 1) * P, :])
        for t in range(MT):
            nc.sync.dma_start(out=x1res[t], in_=f1s[t * P : (t + 1) * P, :])

        # ---- Phase F2: normalize + transpose + AllGather, chunk by chunk ----
        def f2_group(g):
            gsl = slice(g * GT, (g + 1) * GT)
            for t in range(g * GT, (g + 1) * GT):
                sumsq_col(x2res[t], ss2g[:, t : t + 1], "2")
            rsqrt_quarter(inv2g, ss2g, gsl, "2")
            nc.vector.tensor_scalar_mul(i232[:, gsl], inv2g[:, gsl], F2S)
            for t in range(g * GT, (g + 1) * GT):
                x2c = work.tile([P, D], BF16, tag="x2c", name="x2c")
                nc.vector.tensor_scalar_mul(x2c, x2res[t], i232[:, t : t + 1])
                transpose_tile(x2c, f2T[g], (t - g * GT) * P)
            for c in range(DC):
                nc.sync.dma_start(
                    out=agin[g][c * P : (c + 1) * P, :], in_=f2T[g][:, c, :]
                )
            nc.gpsimd.collective_compute(
                "AllGather",
                ALU.bypass,
                replica_groups=[list(range(N_CORES))],
                ins=[agin[g][:, :].opt()],
                outs=[agout[g][:, :].opt()],
            )

        for g in range(NG):
            f2_group(g)

        # ---- Phase F1: cast, transpose, stats, diag -------------------------
        def f1_group(g):
            gsl = slice(g * GT, (g + 1) * GT)
            for t in range(g * GT, (g + 1) * GT):
                x1c = work.tile([P, D], BF16, tag="x1c", name="x1c")
                nc.vector.tensor_copy(x1c, x1res[t])
                transpose_tile(x1c, f1p, t * P)
                sumsq_col(x1res[t], ss1g[:, t : t + 1], "1")
                # diag via polarization: 2*x1.x2 = ||x1+x2||^2 - ||x1||^2 - ||x2||^2
                x12 = work.tile([P, D], F32, tag="x12", bufs=2, name="x12")
                nc.vector.tensor_tensor(x12, x1res[t], x2res[t], ALU.add)
                sumsq_col(x12, ss12g[:, t : t + 1], "12")
            rsqrt_quarter(inv1g, ss1g, gsl, "1")
            nc.vector.tensor_scalar_mul(
                scale1g[:, gsl], inv1g[:, gsl], inv_temp / F2S
            )
            # diag_raw = 0.5*(ss12 - ss1 - ss2); diag = diag_raw * inv1 * inv2.
            nc.vector.tensor_tensor(
                diag_raw[:, gsl], ss12g[:, gsl], ss1g[:, gsl], ALU.subtract
            )
            nc.vector.tensor_tensor(
                diag_raw[:, gsl], diag_raw[:, gsl], ss2g[:, gsl], ALU.subtract
            )
            nc.vector.tensor_tensor(
                diag[:, gsl], diag_raw[:, gsl], inv1g[:, gsl], ALU.mult
            )
            nc.vector.tensor_tensor(
                diag[:, gsl], diag[:, gsl], inv2g[:, gsl], ALU.mult
            )
            nc.vector.tensor_scalar_mul(diag[:, gsl], diag[:, gsl], 0.5)

        for g in range(NG):
            f1_group(g)

        # ---- Phase 3: fused logits -> exp(scale=inv1/(T*32)) -> row-sums ----
        def main_group(g):
            for h in range(2):
                rh = rhsp.tile([P, DC, EXPW], F8, tag="rh", name="rh")
                for j in range(4):
                    r = 4 * h + j
                    for c in range(DC):
                        nc.sync.dma_start(
                            out=rh[:, c, j * GW : (j + 1) * GW],
                            in_=agout[g][r * D + c * P : r * D + (c + 1) * P, :],
                        )
                for mt in range(MT):
                    ps = psum.tile([P, EXPW], F32, tag="ps", name="ps")
                    for cp in range(2):
                        lhsT = f1p[:, 2 * cp : 2 * cp + 2, mt * P : (mt + 1) * P]
                        for j in range(4):
                            nc.tensor.matmul(
                                ps[:, j * GW : (j + 1) * GW],
                                lhsT=lhsT,
                                rhs=rh[:, 2 * cp : 2 * cp + 2, j * GW : (j + 1) * GW],
                                start=(cp == 0),
                                stop=(cp == 1),
                                perf_mode=DR,
                            )
                    # Row-sum on DVE (otherwise idle in the main phase)
                    # instead of the ACT accumulator drain, which costs 182ns
                    # on the pacing engine.
                    ex = work.tile([P, EXPW], BF16, tag="ex", bufs=4, name="ex")
                    nc.scalar.activation(
                        ex, ps, AF.Exp, scale=scale1g[:, mt : mt + 1]
                    )
                    nc.vector.reduce_sum(
                        rowsums[:, mt, 2 * g + h : 2 * g + h + 1],
                        ex,
                        axis=mybir.AxisListType.X,
                    )

        for g in range(NG):
            main_group(g)

        # ---- Phase 4: logsumexp, subtract diag, reduce -----------------------
        s16 = stats.tile([P, MT], F32, tag="s16", name="s16")
        nc.vector.reduce_sum(s16, rowsums, axis=mybir.AxisListType.X)
        lse16 = stats.tile([P, MT], F32, tag="lse16", name="lse16")
        nc.scalar.activation(lse16, s16, AF.Ln)
        # losses = lse - diag/T = (diag * -1/T) + lse
        nc.vector.scalar_tensor_tensor(
            out=losses,
            in0=diag,
            scalar=-inv_temp,
            in1=lse16,
            op0=ALU.mult,
            op1=ALU.add,
        )

        loss_col = stats.tile([P, 1], F32, tag="lc", name="loss_col")
        nc.vector.reduce_sum(loss_col, losses, axis=mybir.AxisListType.X)
        fin = psum.tile([1, 1], F32, tag="ps", name="fin")
        nc.tensor.matmul(fin, lhsT=loss_col, rhs=ones_col, start=True, stop=True)
        res = stats.tile([1, 1], F32, tag="res", name="res")
        nc.any.tensor_copy(res, fin)
        nc.sync.dma_start(out=out[:, :], in_=res)

    return nc


_WAIT_EXEMPT = ("InstCall",)


def _legalize_sync_waits(nc, limit=1):
    """Walrus codegen rejects instructions carrying more than ~1 embedded
    semaphore wait ("Too many sync wait commands"). Move excess waits onto
    injected same-engine NoOps (one wait each) ahead of the instruction —
    semantically identical (the engine blocks on the NoOps first)."""
    n_split = 0
    for b in nc.m.functions[0].blocks:
        insts = b.instructions
        out = []
        changed = False
        for ins in insts:
            si = ins.sync_info
            tname = type(ins).__name__
            if (
                si is not None
                and len(si.on_wait) > limit
                and tname not in _WAIT_EXEMPT
            ):
                waits = list(si.on_wait)
                keep, excess = waits[:limit], waits[limit:]
                for j, w in enumerate(excess):
                    noop = mybir.InstNoOp(name=f"{ins.name}-ws{j}", ins=[], outs=[])
                    noop.engine = ins.engine
                    noop.sync_info = mybir.SyncInfo(on_wait=[w], on_update=[])
                    out.append(noop)
                ins.sync_info = mybir.SyncInfo(
                    on_wait=keep, on_update=list(si.on_update)
                )
                n_split += 1
                changed = True
            out.append(ins)
        if changed:
            b.instructions = out
    return n_split


def _maybe_patch_ldw_opt():
    """KERNEL_LDW_OPT=1 flips walrus --enable-ldw-opt to true (dedupes /
    optimizes LDWEIGHTS); A/B experiment, correctness-checked by the rel-err
    gate."""
    if not int(os.environ.get("KERNEL_LDW_OPT", "0")):
        return
    import concourse.bass_utils as bu

    if getattr(bu.run_command, "_ldw_patched", False):
        return
    orig = bu.run_command

    def run2(cmd, **kw):
        cmd = [
            "--enable-ldw-opt=true" if c == "--enable-ldw-opt=false" else c
            for c in cmd
        ]
        return orig(cmd, **kw)

    run2._ldw_patched = True
    bu.run_command = run2


@lru_cache(maxsize=1)
def _get_nc():
    _maybe_patch_ldw_opt()
    nc = _build_bass()
    _legalize_sync_waits(nc)
    return nc


def kernel(features1, features2):
    global LAST_EXEC_TIME_NS
    f1 = np.ascontiguousarray(np.asarray(features1, dtype=np.float32))
    f2 = np.ascontiguousarray(np.asarray(features2, dtype=np.float32))
    assert f1.shape == (N, D) and f2.shape == (N, D)

    in_maps = []
    for i in range(N_CORES):
        sl = slice(i * M_LOCAL, (i + 1) * M_LOCAL)
        in_maps.append(
            {
                "f1s": np.ascontiguousarray(f1[sl]),
                "f2o": np.ascontiguousarray(f2[sl]),
            }
        )

    nc = _get_nc()
    trace = bool(int(os.environ.get("KERNEL_TRACE", "0")))
    if trace:
        _install_ntff_hook()
    tmpdir = os.environ.get("KERNEL_TRACE_DIR") or None
    r = run_bass_kernel_spmd(
        nc, in_maps, list(range(N_CORES)), trace=trace, tmpdir=tmpdir
    )
    LAST_EXEC_TIME_NS = r.exec_time_ns

    total = sum(float(r.results[i]["out"][0, 0]) for i in range(N_CORES))
    return np.float32(total / N)
